# revision 11
# baseline (speedup 1.0000x reference)
"""2D DCT-II (4096x4096, fp32) on 8 TRN2 NeuronCores.

out = C0 @ x @ C1^T with C0 = C1 = C, C[k, i] = cos(pi*(2i+1)*k/(2N)).

Fast-DCT folding via the basis reflection symmetries
    C[u, N-1-i]   = (-1)^u      * C[u, i]     (level 1, both axes)
    C[v, N/2-1-j] = (-1)^(v/2)  * C[v, j]     (v%2==0; level 2, columns)
    C[v, N/4-1-j] = (-1)^(v/4)  * C[v, j]     (v%4==0; level 3, columns)

level 1 (both stages, folded on the HOST -> half FLOPs + half HBM):
  - cores 0-3 own the even output rows u, cores 4-7 the odd rows;
  - host supplies doubly-folded x quarters xa/xb [2048,2048] (feeding
    even-v / odd-v outputs) and basis slices;
levels 2+3 (column axis; even v split into v%4==2, v%8==0, v%8==4):
  - xa arrives with its columns permuted so that each fold level pairs
    reflection partners at identical partition offsets; two short DVE
    butterflies (8+4 tile ops, running under stage 1's remaining
    matmuls) then yield the quarter-/eighth-folded intermediates, and
    the even-v sections contract over only 1024 / 512 elements.

Device pipeline per core (all matmuls fp32r = full-rate FP22):
  stage 1: T(E|O)^T[j', m] = sum_i' x(a|b)[i', j'] * c0tp[i', m]
     lhsT = x tile (streamed, 1 MB DMAs), rhs = c0tp (SBUF-resident)
     -> 512 matmuls; intermediates land transposed in SBUF, exactly the
     stationary layout stage 2 needs.
  butterflies: t2e/t2o = TE'[:Q] -+ TE'[Q:];  t3e/t3o = t2e[:E] -+ t2e[E:]
  stage 2: v%8==0: sum_{j<512}  t3e^T * C[8v, j]     (16 matmuls)
           v%8==4: sum_{j<512}  t3o^T * C[8v+4, j]   (16 matmuls)
           v%4==2: sum_{j<1024} t2o^T * C[4v+2, j]   (64 matmuls)
           v odd:  sum_{j<2048} TO^T  * C[2v+1, j]   (256 matmuls)
     rhs = basis (streamed), lhsT = intermediates (SBUF-resident).
  Output leaves in section-packed columns [v80|v84|v2|vodd]; the host
  de-interleaves (pure numpy slicing).

PSUM: 4-bank accumulation groups alternate between two bank sets so a
group's drain (DVE/ACT copies, alternating) overlaps the next group's
matmuls; junk-matmul warm-up + fillers keep the PE's HAM clock gate at
2.4 GHz through the DMA-starved opening. Total per-core: 864 matmuls
(~196 us PE) + ~58 MB HBM.
"""

import math

import numpy as np

import concourse.mybir as mybir
import concourse.tile as tile
from concourse import bacc
from concourse.bass_utils import run_bass_kernel_spmd

N = 4096
H = N // 2  # 2048: level-1 folded contraction
Q = N // 4  # 1024: level-2 folded contraction
E = N // 8  # 512:  level-3 folded contraction
P = 128
HT = H // P  # 16
QT = Q // P  # 8
ET = E // P  # 4
NCORES = 8
RB = 512  # output rows per core
G = 512  # column-group / matmul moving width
KQ = 4  # k-tiles per streaming DMA (1 MB)

f32 = mybir.dt.float32
f32r = mybir.dt.float32r

_CACHE = {}


def _build():
    nc = bacc.Bacc("TRN2", target_bir_lowering=False, debug=False)
    xa_d = nc.dram_tensor("xa", [H, H], f32r, kind="ExternalInput")
    xb_d = nc.dram_tensor("xb", [H, H], f32r, kind="ExternalInput")
    c0tp_d = nc.dram_tensor("c0tp", [H, RB], f32r, kind="ExternalInput")
    c1v8_d = nc.dram_tensor("c1v8", [E, Q], f32r, kind="ExternalInput")
    c1v2_d = nc.dram_tensor("c1v2", [Q, Q], f32r, kind="ExternalInput")
    c1vo_d = nc.dram_tensor("c1vo", [H, H], f32r, kind="ExternalInput")
    out_d = nc.dram_tensor("out", [RB, N], f32, kind="ExternalOutput")

    state = {"ggc": 0}

    with tile.TileContext(nc) as tc:
        with (
            tc.tile_pool(name="persist", bufs=1) as persist,
            tc.tile_pool(name="xin", bufs=4) as xin,
            tc.tile_pool(name="cin", bufs=4) as cin,
            tc.tile_pool(name="osb", bufs=3) as osb,
            tc.tile_pool(name="ps", bufs=1, space="PSUM") as ps,
        ):
            c0tp_sb = persist.tile([P, HT, RB], f32r, tag="c0", name="c0tp_sb")
            # TE' (permuted) / TO intermediates: [j', m] as [128, 16, 512]
            t_sb = [
                persist.tile([P, HT, RB], f32r, tag=f"t{h}", name=f"t{h}_sb")
                for h in range(2)
            ]
            # odd-sign butterfly outputs (even-sign halves fold in place)
            t2o_sb = persist.tile([P, QT, RB], f32r, tag="t2o", name="t2o_sb")
            t3o_sb = persist.tile([P, ET, RB], f32r, tag="t3o", name="t3o_sb")

            def banks(n=4):
                g = state["ggc"]
                state["ggc"] += 1
                return [
                    ps.tile(
                        [P, G], f32, tag=f"ps{(g % 2) * 4 + i}",
                        name=f"ps{(g % 2) * 4 + i}",
                    )
                    for i in range(n)
                ]

            def drain(bk, mb, dst):
                # alternate DVE/ACT so section-end drains parallelize
                if mb % 2 == 0:
                    nc.vector.tensor_copy(dst, bk[:])
                else:
                    nc.scalar.copy(dst, bk[:])

            # PE warm-up: the HAM clock gate needs ~3.4 us of sustained
            # matmul activity to lift the PE from 1.2 to 2.4 GHz, and the
            # first real matmul can't start until ~0.5 MB of operands
            # land (~10 us incl. preamble). Chew zeros meanwhile so the
            # real stream starts warm.
            junk = persist.tile([P, P], f32, tag="junk", name="junk")
            nc.gpsimd.memset(junk[:], 0)
            jps = ps.tile([P, P], f32, tag="ps7", name="jps")
            for _ in range(32):
                nc.tensor.matmul(jps[:], junk[:], junk[:], start=True, stop=True)

            # ---- stage 1: T(E|O)^T[j', m] = sum_i' x(a|b)[i',j'] c0tp[i',m]
            for h in range(2):
                src = xa_d if h == 0 else xb_d
                for g in range(4):  # j'-column groups of 512
                    bk = banks()
                    for kq in range(HT // KQ):
                        if h == 0 and g == 0 and kq == 0:
                            # fine-grained first chunk: first matmuls can
                            # start after ~512 KB instead of 2 MB
                            for ko in range(KQ):
                                nc.scalar.dma_start(
                                    c0tp_sb[:, ko, :],
                                    c0tp_d[ko * P:(ko + 1) * P, :],
                                )
                                if ko == 0:
                                    xt = xin.tile(
                                        [P, KQ, G], f32r, tag="xt", name="xt"
                                    )
                                nc.sync.dma_start(
                                    xt[:, ko, :],
                                    src[ko * P:(ko + 1) * P, 0:G],
                                )
                        else:
                            if h == 0 and g == 0:
                                nc.scalar.dma_start(
                                    c0tp_sb[:, kq * KQ:(kq + 1) * KQ, :],
                                    c0tp_d[
                                        kq * KQ * P:(kq + 1) * KQ * P, :
                                    ].rearrange("(o p) m -> p o m", p=P),
                                )
                            xt = xin.tile([P, KQ, G], f32r, tag="xt", name="xt")
                            nc.sync.dma_start(
                                xt[:],
                                src[
                                    kq * KQ * P:(kq + 1) * KQ * P,
                                    g * G:(g + 1) * G,
                                ].rearrange("(o p) n -> p o n", p=P),
                            )
                        for ko in range(KQ):
                            it = kq * KQ + ko
                            for jb in range(4):
                                nc.tensor.matmul(
                                    bk[jb][:],
                                    xt[:, ko, jb * P:(jb + 1) * P],
                                    c0tp_sb[:, it, :],
                                    start=(it == 0),
                                    stop=(it == HT - 1),
                                )
                        if h == 0 and g < 2 and (g > 0 or kq >= 2):
                            # idle-bank warm fillers across the early
                            # HBM-starved chunk boundaries
                            ftag = "ps4" if g == 0 else "ps0"
                            fps = ps.tile(
                                [P, P], f32, tag=ftag, name="fps"
                            )
                            for _ in range(2):
                                nc.tensor.matmul(
                                    fps[:], junk[:], junk[:],
                                    start=True, stop=True,
                                )
                    for jb in range(4):
                        nc.vector.tensor_copy(
                            t_sb[h][:, g * 4 + jb, :], bk[jb][:]
                        )
                if h == 0:
                    # column-fold butterflies on TE' (DVE work overlapping
                    # the TO-half matmuls); partner tiles are partition-
                    # aligned thanks to the host column permutation.
                    for jt in range(QT):  # level 2
                        nc.vector.tensor_tensor(
                            t2o_sb[:, jt, :],
                            t_sb[0][:, jt, :],
                            t_sb[0][:, QT + jt, :],
                            mybir.AluOpType.subtract,
                        )
                        nc.vector.tensor_tensor(
                            t_sb[0][:, jt, :],
                            t_sb[0][:, jt, :],
                            t_sb[0][:, QT + jt, :],
                            mybir.AluOpType.add,
                        )
                    for jt in range(ET):  # level 3 (on the level-2 evens)
                        nc.vector.tensor_tensor(
                            t3o_sb[:, jt, :],
                            t_sb[0][:, jt, :],
                            t_sb[0][:, ET + jt, :],
                            mybir.AluOpType.subtract,
                        )
                        nc.vector.tensor_tensor(
                            t_sb[0][:, jt, :],
                            t_sb[0][:, jt, :],
                            t_sb[0][:, ET + jt, :],
                            mybir.AluOpType.add,
                        )

            # ---- stage 2 ----
            # v%8==0 / v%8==4: 512-deep contraction, one 512-col block each
            for sec in range(2):
                lhs = t_sb[0] if sec == 0 else t3o_sb
                bk = banks()
                ct = cin.tile([P, KQ, G], f32r, tag="ct", name="ct")
                nc.sync.dma_start(
                    ct[:],
                    c1v8_d[:, sec * G:(sec + 1) * G].rearrange(
                        "(o p) v -> p o v", p=P
                    ),
                )
                for jt in range(ET):
                    for mb in range(4):
                        nc.tensor.matmul(
                            bk[mb][:],
                            lhs[:, jt, mb * P:(mb + 1) * P],
                            ct[:, jt, :],
                            start=(jt == 0),
                            stop=(jt == ET - 1),
                        )
                for mb in range(4):
                    ot = osb.tile([P, G], f32, tag="ot", name="ot")
                    drain(bk[mb], mb, ot[:])
                    nc.gpsimd.dma_start(
                        out_d[mb * P:(mb + 1) * P, sec * G:(sec + 1) * G],
                        ot[:],
                    )
            # v%4==2: 1024-deep contraction over t2o (basis rows arrive
            # pre-reordered to match the permuted j'' layout)
            for blk in range(2):
                bk = banks()
                for jq in range(QT // KQ):
                    ct = cin.tile([P, KQ, G], f32r, tag="ct", name="ct")
                    nc.sync.dma_start(
                        ct[:],
                        c1v2_d[
                            jq * KQ * P:(jq + 1) * KQ * P,
                            blk * G:(blk + 1) * G,
                        ].rearrange("(o p) v -> p o v", p=P),
                    )
                    for jo in range(KQ):
                        jt = jq * KQ + jo
                        for mb in range(4):
                            nc.tensor.matmul(
                                bk[mb][:],
                                t2o_sb[:, jt, mb * P:(mb + 1) * P],
                                ct[:, jo, :],
                                start=(jt == 0),
                                stop=(jt == QT - 1),
                            )
                for mb in range(4):
                    ot = osb.tile([P, G], f32, tag="ot", name="ot")
                    drain(bk[mb], mb, ot[:])
                    nc.gpsimd.dma_start(
                        out_d[
                            mb * P:(mb + 1) * P,
                            Q + blk * G:Q + (blk + 1) * G,
                        ],
                        ot[:],
                    )
            # v odd: 2048-deep contraction over TO
            for vg in range(4):
                bk = banks()
                for jq in range(HT // KQ):
                    ct = cin.tile([P, KQ, G], f32r, tag="ct", name="ct")
                    nc.sync.dma_start(
                        ct[:],
                        c1vo_d[
                            jq * KQ * P:(jq + 1) * KQ * P,
                            vg * G:(vg + 1) * G,
                        ].rearrange("(o p) v -> p o v", p=P),
                    )
                    for jo in range(KQ):
                        jt = jq * KQ + jo
                        for mb in range(4):
                            nc.tensor.matmul(
                                bk[mb][:],
                                t_sb[1][:, jt, mb * P:(mb + 1) * P],
                                ct[:, jo, :],
                                start=(jt == 0),
                                stop=(jt == HT - 1),
                            )
                for mb in range(4):
                    ot = osb.tile([P, G], f32, tag="ot", name="ot")
                    drain(bk[mb], mb, ot[:])
                    if vg == 3:
                        eng = nc.sync if mb % 2 == 0 else nc.scalar
                    else:
                        eng = nc.gpsimd
                    eng.dma_start(
                        out_d[
                            mb * P:(mb + 1) * P,
                            2048 + vg * G:2048 + (vg + 1) * G,
                        ],
                        ot[:],
                    )
    nc.compile()
    return nc


def _get_nc():
    if "nc" not in _CACHE:
        _CACHE["nc"] = _build()
    return _CACHE["nc"]


def _dct_basis_t():
    """C^T as float32 [N, N]: C^T[i, k] = cos(pi*(2i+1)*k/(2N)).

    Matches the reference's float32 jnp computation (fp32 argument
    arithmetic) so basis rounding does not diverge from the oracle."""
    if "ct" in _CACHE:
        return _CACHE["ct"]
    ct = None
    try:
        import jax
        import jax.numpy as jnp

        cpus = jax.devices("cpu")
        with jax.default_device(cpus[0]):
            k = jnp.arange(N, dtype=jnp.float32)[:, None]
            i = jnp.arange(N, dtype=jnp.float32)[None, :]
            c = jnp.cos((jnp.pi / (2.0 * N)) * (2.0 * i + 1.0) * k)
            ct = np.ascontiguousarray(np.asarray(c).T)
    except Exception:
        pass
    if ct is None:
        k = np.arange(N, dtype=np.float32)[:, None]
        i = np.arange(N, dtype=np.float32)[None, :]
        s = math.pi / (2.0 * N)
        arg = (s * (2.0 * i + 1.0)).astype(np.float32) * k
        ct = np.ascontiguousarray(np.cos(arg.astype(np.float32)).T)
    _CACHE["ct"] = ct
    return ct


# xa column permutation: level-3-ready order inside each level-2 half
_IDX3 = np.concatenate([np.arange(E), np.arange(Q - 1, E - 1, -1)])
_PERM = np.concatenate([_IDX3, (H - 1) - _IDX3])


def _in_maps(x):
    x = np.asarray(x, dtype=np.float32)
    ct = _dct_basis_t()

    # level-1 host folds (exact up to fp32 rounding)
    xE = x[:H] + x[:H - 1:-1]
    xO = x[:H] - x[:H - 1:-1]
    quads = {}
    for tag, xf in (("E", xE), ("O", xO)):
        xa = xf[:, :H] + xf[:, :H - 1:-1]
        quads[tag + "a"] = np.ascontiguousarray(xa[:, _PERM])
        quads[tag + "b"] = np.ascontiguousarray(xf[:, :H] - xf[:, :H - 1:-1])

    # stage-2 bases
    c1v8 = np.empty((E, Q), dtype=np.float32)
    c1v8[:, :G] = ct[:E, 0::8]  # C[8v, j]^T rows j<512
    c1v8[:, G:] = ct[:E, 4::8]
    c1v2 = np.ascontiguousarray(ct[:Q, 2::4][_IDX3, :])
    c1vo = np.ascontiguousarray(ct[:H, 1::2])

    maps = []
    for c in range(NCORES):
        par = 0 if c < 4 else 1
        base = 1024 * (c % 4)
        maps.append(
            {
                "xa": quads[("E" if par == 0 else "O") + "a"],
                "xb": quads[("E" if par == 0 else "O") + "b"],
                "c0tp": np.ascontiguousarray(
                    ct[:H, base + par:base + 1024 + par:2]
                ),
                "c1v8": c1v8,
                "c1v2": c1v2,
                "c1vo": c1vo,
            }
        )
    return maps


def _assemble(results):
    full = np.empty((N, N), dtype=np.float32)
    for c in range(NCORES):
        par = 0 if c < 4 else 1
        base = 1024 * (c % 4)
        rows = full[base + par:base + 1024 + par:2]
        dev = results[c]["out"]
        rows[:, 0::8] = dev[:, 0:512]
        rows[:, 4::8] = dev[:, 512:1024]
        rows[:, 2::4] = dev[:, 1024:2048]
        rows[:, 1::2] = dev[:, 2048:4096]
    return full


def _run(x, **kwargs):
    nc = _get_nc()
    in_maps = _in_maps(x)
    last = None
    for attempt in range(3):
        try:
            res = run_bass_kernel_spmd(
                nc, in_maps, core_ids=list(range(NCORES)), **kwargs
            )
            return _assemble(res.results), res
        except Exception as e:  # transient NRT/device faults happen rarely
            last = e
    raise last


def kernel(x):
    out, _ = _run(x)
    return out
